# revision 2
# baseline (speedup 1.0000x reference)
"""Trainium2 Bass kernel for the conditional-prior VAE rational-quadratic
spline loss head — v3: mixed fp32/fp16, staged tail.

Per-tile stage A computes the per-bin work (exp tables, fp32 knot cumsum,
fp32 residual build, fp16 searchsorted flags + one-hot, fp16 gather
products + shared fold tree) and stages the 8 gathered quantities plus a
few per-row fp32 helpers into core-wide buffers. Stage B then evaluates
the whole rational-quadratic tail once over all rows, so the ~35 tiny
[P, rows, 2] ops pay instruction overhead once instead of once per tile.

Row mapping: row = p*(NT*T) + it*T + t (partition-major), so the staged
output is one contiguous DMA per core.
"""

import sys

for _p in ("/opt/trn_rl_repo",):
    if _p not in sys.path:
        sys.path.insert(0, _p)

from contextlib import ExitStack

import numpy as np

import concourse.bass as bass
import concourse.tile as tile
from concourse import bacc, mybir
from concourse.bass_utils import run_bass_kernel_spmd

_AF = mybir.ActivationFunctionType
_KERNEL_FUNCS = {_AF.Exp, _AF.Ln, _AF.Abs, _AF.Copy, _AF.Identity}
_ONE_TABLE = "natural_log_exp_and_others"
_orig_get_act_tables = bacc.get_activation_tables


def _patched_act_tables(arch):
    out = {}
    for name, funcs in _orig_get_act_tables(arch).items():
        if name == _ONE_TABLE:
            out[name] = funcs
        else:
            out[name] = funcs - _KERNEL_FUNCS
    return out


bacc.get_activation_tables = _patched_act_tables

F32 = mybir.dt.float32
F16 = mybir.dt.float16
OP = mybir.AluOpType
AF = mybir.ActivationFunctionType

TAIL = 3.5
MINB = 0.01
MIND = 0.01
NB = 8
SW = 2.0 * TAIL - MINB * NB  # 6.92

N_TOTAL = 524288
N_CORES = 8
NC_ROWS = N_TOTAL // N_CORES  # 65536 rows per core
T_DEF = 128                   # rows per partition per tile

_POOL_F16 = True   # allow Pool to run fp16 tensor_tensor (set False if HW balks)


def build_program(nc_rows: int = NC_ROWS, T: int = T_DEF, repeat: int = 1):
    P = 128
    R = P * T
    assert nc_rows % R == 0
    NT = nc_rows // R           # tiles per pass
    TT_ = NT * T                # staged rows per partition

    nc = bacc.Bacc("TRN2", target_bir_lowering=False, debug=False)

    p_dram = nc.dram_tensor("out_params", [nc_rows, 48], F32, kind="ExternalInput")
    e_dram = nc.dram_tensor("eps", [nc_rows, 2], F32, kind="ExternalInput")
    y_dram = nc.dram_tensor("y", [nc_rows, 2], F32, kind="ExternalOutput")

    # DRAM views in partition-major row order: row = p*TT_ + trow
    p_v = p_dram.rearrange("(p t) c -> p t c", p=P)   # [P, TT_, 48]
    e_v = e_dram.rearrange("(p t) c -> p t c", p=P)
    y_v = y_dram.rearrange("(p t) c -> p t c", p=P)

    kvec_dram = nc.inline_tensor(
        (np.arange(9, dtype=np.float32) * MINB).reshape(9), name="kvec9"
    )

    A = nc.scalar
    V = nc.vector
    GP = nc.gpsimd
    GPF = GP if _POOL_F16 else V
    SY = nc.sync

    def stt(eng, out, in0, s, in1, op0, op1):
        return eng.scalar_tensor_tensor(
            out=out, in0=in0, scalar=float(s), in1=in1, op0=op0, op1=op1
        )

    def tt(eng, out, in0, in1, op):
        return eng.tensor_tensor(out=out, in0=in0, in1=in1, op=op)

    def ts(out, in0, s1, op0, s2=None, op1=None):
        kw = {}
        if op1 is not None:
            kw["op1"] = op1
        return V.tensor_scalar(out=out, in0=in0, scalar1=s1, scalar2=s2,
                               op0=op0, **kw)

    with tile.TileContext(nc) as tc, ExitStack() as ctx:
        io_pool = ctx.enter_context(tc.tile_pool(name="io", bufs=2))
        cpool = ctx.enter_context(tc.tile_pool(name="cpool", bufs=2))
        spool = ctx.enter_context(tc.tile_pool(name="spool", bufs=1))
        stage = ctx.enter_context(tc.tile_pool(name="stage", bufs=1))
        ones = ctx.enter_context(tc.tile_pool(name="ones", bufs=1))

        kvec9 = ones.tile([P, 9], F32)
        SY.dma_start(
            out=kvec9,
            in_=bass.AP(tensor=kvec_dram, offset=0, ap=[[0, P], [1, 9]]),
        )

        NST = 2 if NT % 2 == 0 and NT >= 2 else 1   # super-tiles per pass
        TB = NT // NST                               # tiles per super-tile
        TBT = TB * T                                 # rows/partition per super-tile

        def _super(st):
            SG = stage.tile([P, TBT, 8, 2], F16, tag="SG", bufs=2)
            XMst = stage.tile([P, TBT, 2], F32, tag="XMst", bufs=2)
            MMst = stage.tile([P, TBT, 2], F16, tag="MMst", bufs=2)
            XYLst = stage.tile([P, TBT, 2], F32, tag="XYLst", bufs=2)

            for it in range(TB):
                gl = slice((st * TB + it) * T, (st * TB + it + 1) * T)
                sl = slice(it * T, (it + 1) * T)

                pt = io_pool.tile([P, T, 48], F32)
                et = io_pool.tile([P, T, 2], F32)
                SY.dma_start(out=pt, in_=p_v[:, gl, :])
                SY.dma_start(out=et, in_=e_v[:, gl, :])

                # ---- ACT: transcendentals -----------------------------
                Gw = cpool.tile([P, T, 8, 2], F32, tag="Gw")
                A.activation(
                    out=Gw.rearrange("p t b j -> p t j b"),
                    in_=pt[:, :, 2:18], func=AF.Exp,
                )
                Gh = cpool.tile([P, T, 8, 2], F16, tag="Gh")
                A.activation(
                    out=Gh.rearrange("p t b j -> p t j b"),
                    in_=pt[:, :, 18:34], func=AF.Exp,
                )
                Dpad = cpool.tile([P, T, 9, 2], F16, tag="Dpad")
                Dview = Dpad[:, :, 1:8].rearrange("p t b j -> p t j b")
                A.activation(out=Dview, in_=pt[:, :, 34:48], func=AF.Exp)
                A.activation(out=Dview, in_=Dview, func=AF.Ln, bias=1.0)
                GP.memset(Dpad[:, :, 0], 1.0 - MIND)
                GP.memset(Dpad[:, :, 8], 1.0 - MIND)

                Mabs = spool.tile([P, T, 2], F32, tag="Mabs")
                A.activation(out=Mabs, in_=et, func=AF.Abs)

                # ---- width-path cumsum (fp32) -------------------------
                E = cpool.tile([P, T, 9, 2], F32, tag="E")
                GP.memset(E[:, :, 0], 0.0)
                ts(out=E[:, :, 1], in0=Gw[:, :, 0], s1=1.0, op0=OP.mult)
                for k in range(2, 9):
                    stt(V, E[:, :, k], Gw[:, :, k - 1], 1.0, E[:, :, k - 1],
                        OP.mult, OP.add)
                sw = E[:, :, 8]

                # ---- mask / shifted x / SX ----------------------------
                mm = MMst[:, sl]
                ts(out=mm, in0=Mabs, s1=TAIL, op0=OP.is_lt)
                xm = XMst[:, sl]
                tt(V, xm, et, mm, OP.mult)
                ts(out=xm, in0=xm, s1=TAIL, op0=OP.add)
                invSw = spool.tile([P, T, 2], F32, tag="invSw")
                rsc = spool.tile([P, T, 2], F32, tag="rsc")
                swc = spool.tile([P, T, 2], F32, tag="swc")
                ts(out=swc, in0=sw, s1=1.0, op0=OP.mult)
                V.reciprocal_approx_accurate(
                    out=invSw.rearrange("p t j -> p (t j)"),
                    in_=swc.rearrange("p t j -> p (t j)"),
                    scratch=rsc.rearrange("p t j -> p (t j)"),
                )

                # ---- residuals R9[k] = (xt - MINB*k) - SW*E[k]/s_w ----
                # XK (Pool) depends only on xt: early, off the DVE spine.
                KS = spool.tile([P, T, 9, 2], F32, tag="KS")
                kb = kvec9.unsqueeze(1).unsqueeze(3).broadcast_to([P, T, 9, 2])
                xmb = xm.unsqueeze(2).broadcast_to([P, T, 9, 2])
                tt(GP, KS, xmb, kb, OP.subtract)            # KS := XK
                EI = spool.tile([P, T, 9, 2], F32, tag="EI")
                iwb = invSw.unsqueeze(2).broadcast_to([P, T, 9, 2])
                tt(V, EI, E, iwb, OP.mult)
                R9 = cpool.tile([P, T, 9, 2], F16, tag="R9")
                stt(V, R9, EI, -SW, KS, OP.mult, OP.add)

                # ---- flags + one-hot (fp16) ---------------------------
                OHGE = spool.tile([P, T, 2, 9, 2], F16, tag="OHGE")
                GE = OHGE[:, :, 0]
                OH = OHGE[:, :, 1, 0:8]
                GP.memset(GE[:, :, 0], 1.0)
                ts(out=GE[:, :, 1:9], in0=R9[:, :, 1:9], s1=0.0, op0=OP.is_ge)
                tt(V, OH, GE[:, :, 0:8], GE[:, :, 1:9], OP.subtract)
                ge19 = GE[:, :, 1:9]

                # ---- gather products + fold tree (fp16) ---------------
                # q: 0 n, 1 nR, 2 hg, 3 Ah, 4 d0g, 5 d1g, 6 MB, 7 s_h
                PR = spool.tile([P, T, 4, 8, 2], F16, tag="PR")
                tt(V, PR[:, :, 0], OH, R9[:, :, 0:8], OP.mult)
                tt(V, PR[:, :, 1], OH, R9[:, :, 1:9], OP.mult)
                tt(V, PR[:, :, 2], OH, Gh, OP.mult)
                tt(GPF, PR[:, :, 3], ge19, Gh, OP.mult)
                tt(V, PR[:, :, :, 0:4], PR[:, :, :, 0:4], PR[:, :, :, 4:8],
                   OP.add)
                tt(V, PR[:, :, :, 0:2], PR[:, :, :, 0:2], PR[:, :, :, 2:4],
                   OP.add)
                tt(V, SG[:, sl, 0:4], PR[:, :, :, 0], PR[:, :, :, 1], OP.add)

                PRb = spool.tile([P, T, 4, 8, 2], F16, tag="PR")
                tt(V, PRb[:, :, 0], OH, Dpad[:, :, 0:8], OP.mult)
                tt(V, PRb[:, :, 1], OH, Dpad[:, :, 1:9], OP.mult)
                ts(out=PRb[:, :, 2], in0=ge19, s1=MINB, op0=OP.mult)
                A.activation(out=PRb[:, :, 3], in_=Gh, func=AF.Copy)
                tt(V, PRb[:, :, :, 0:4], PRb[:, :, :, 0:4], PRb[:, :, :, 4:8],
                   OP.add)
                tt(V, PRb[:, :, :, 0:2], PRb[:, :, :, 0:2], PRb[:, :, :, 2:4],
                   OP.add)
                tt(V, SG[:, sl, 4:8], PRb[:, :, :, 0], PRb[:, :, :, 1], OP.add)

                XYL = XYLst[:, sl]
                tt(GP, XYL, et, pt[:, :, 0:2], OP.add)

            # ================= stage B: tail over all rows =============
            SGv = SG
            n_ = SGv[:, :, 0]
            nR = SGv[:, :, 1]
            hg = SGv[:, :, 2]
            Ah = SGv[:, :, 3]
            d0g = SGv[:, :, 4]
            d1g = SGv[:, :, 5]
            MB = SGv[:, :, 6]
            sh16 = SGv[:, :, 7]

            def tl(name, dt=F16):
                return stage.tile([P, TBT, 2], dt, name=name, tag=name)

            F1 = tl("F1", F32)
            ts(out=F1, in0=sh16, s1=1.0, op0=OP.mult)
            invSh = tl("invSh", F32)
            V.reciprocal_approx_fast(out=invSh, in_=F1)

            TH = n_
            QN = tl("QN")
            tt(V, QN, n_, nR, OP.subtract)

            HH = tl("HH")
            stt(V, HH, hg, SW, invSh, OP.mult, OP.mult)
            ts(out=HH, in0=HH, s1=MINB, op0=OP.add)
            Y0 = tl("Y0")
            stt(V, Y0, Ah, SW, invSh, OP.mult, OP.mult)
            tt(V, Y0, Y0, MB, OP.add)

            D0 = tl("D0")
            ts(out=D0, in0=d0g, s1=MIND, op0=OP.add)
            DS = tl("DS")
            tt(V, DS, d0g, d1g, OP.add)
            ts(out=DS, in0=DS, s1=2.0 * MIND, op0=OP.add)

            OM = tl("OM")
            tt(V, OM, QN, TH, OP.subtract)
            U = tl("U")
            tt(V, U, TH, OM, OP.mult)
            VV = tl("VV")
            tt(V, VV, U, QN, OP.mult)
            tt(V, OM, TH, TH, OP.mult)          # OM := theta^2
            NUM = tl("NUM")
            tt(V, NUM, OM, HH, OP.mult)
            tt(V, D0, D0, VV, OP.mult)          # D0 := d0*v
            tt(V, NUM, NUM, D0, OP.add)
            tt(V, NUM, NUM, HH, OP.mult)

            tt(V, OM, QN, QN, OP.mult)          # OM := qn^2
            DEN = tl("DEN")
            tt(V, DEN, OM, HH, OP.mult)
            tt(V, DS, DS, VV, OP.mult)          # DS := DS*v
            tt(V, DEN, DEN, DS, OP.add)
            tt(V, QN, HH, U, OP.mult)           # QN := h*u
            tt(V, QN, QN, QN, OP.add)           # QN := 2*h*u
            tt(V, DEN, DEN, QN, OP.subtract)

            F2 = tl("F2", F32)
            ts(out=F2, in0=DEN, s1=1.0, op0=OP.mult)
            V.reciprocal_approx_fast(out=F1, in_=F2)   # F1 := 1/den

            RT = tl("RT")
            tt(V, RT, NUM, F1, OP.mult)
            tt(V, Y0, Y0, XMst, OP.subtract)
            tt(V, RT, RT, Y0, OP.add)
            tt(V, RT, RT, MMst, OP.mult)
            OUT = stage.tile([P, TBT, 2], F32, tag="OUT", bufs=2)
            tt(V, OUT, RT, XYLst, OP.add)

            SY.dma_start(out=y_v[:, st * TBT : (st + 1) * TBT, :], in_=OUT)

        def _pass():
            for st in range(NST):
                _super(st)

        if repeat > 1:
            with tc.For_i(0, repeat, 1):
                _pass()
        else:
            _pass()

    return nc


_CACHE = {}


def _get_program(nc_rows, T):
    key = (nc_rows, T)
    if key not in _CACHE:
        nc = build_program(nc_rows, T)
        nc.compile()
        _CACHE[key] = nc
    return _CACHE[key]


def kernel(out_params: np.ndarray, eps: np.ndarray) -> np.ndarray:
    assert out_params.shape == (N_TOTAL, 48), out_params.shape
    assert eps.shape == (N_TOTAL, 2), eps.shape
    out_params = np.ascontiguousarray(out_params, dtype=np.float32)
    eps = np.ascontiguousarray(eps, dtype=np.float32)

    nc = _get_program(NC_ROWS, T_DEF)
    core_ids = list(range(N_CORES))
    in_maps = [
        {
            "out_params": out_params[i * NC_ROWS : (i + 1) * NC_ROWS],
            "eps": eps[i * NC_ROWS : (i + 1) * NC_ROWS],
        }
        for i in core_ids
    ]
    res = run_bass_kernel_spmd(nc, in_maps, core_ids)
    return np.concatenate([r["y"] for r in res.results], axis=0)
